# Initial kernel scaffold
#
"""ECC (edge-conditioned convolution) GNN message passing on 8 NeuronCores.

Strategy
--------
Edges are sorted by destination node (host side) and split into 8
contiguous, segment-aligned shards -- one per core.  Each core runs an
identical Bass program over its shard:

  PE    : h1 = relu(W1 @ eaT); h2 = relu(W2 @ h1);
          theta blocks (8x [128,512] per 512-edge tile) = W3T_b.T @ h2;
          b3 term + selector matmuls reduce over i:
             msg[o,e] = sum_i theta[(o,i),e] * xs[i,e]
  ACT   : PSUM->SBUF evacuation (relu / f32->bf16 casts)
  DVE   : theta * xs_rep elementwise (bf16 2x mode) and a masked prefix
          scan (tensor_tensor_scan) that produces running segment sums.

The scan output [32, E_c] goes back to HBM; the host reads each segment's
last column (positions are known statically from the sort), divides by
degree and applies the final relu.  Because shards are segment aligned no
cross-core reduction is needed.
"""

import math
import sys

import numpy as np

for _p in ("/opt/trn_rl_repo", "/root/.axon_site/_ro/trn_rl_repo"):
    if _p not in sys.path:
        sys.path.insert(0, _p)

import ml_dtypes

import concourse.bass as bass
import concourse.mybir as mybir
import concourse.tile as tile
from concourse import bacc
from concourse.bass_utils import run_bass_kernel_spmd

N_NODES = 25000
N_EDGES = 250000
F_IN = 32
F_OUT = 32
EDGE_DIM = 6
H1, H2 = 64, 128
N_CORES = 8
E_TILE = 512

BF16 = ml_dtypes.bfloat16

_program_cache: dict = {}


def _build_program(
    e_c: int, bench_repeat: int | None = None, has_b3: bool = True
) -> "bass.Bass":
    f32 = mybir.dt.float32
    bf16 = mybir.dt.bfloat16
    n_tiles = e_c // E_TILE

    nc = bacc.Bacc(None, target_bir_lowering=False)

    ea_t_d = nc.declare_dram_parameter("eaT", [EDGE_DIM, e_c], bf16, isOutput=False)
    xs_d = nc.declare_dram_parameter("xsrep", [128, e_c], bf16, isOutput=False)
    mk_d = nc.declare_dram_parameter("mask", [F_OUT, e_c], bf16, isOutput=False)
    w1_d = nc.declare_dram_parameter("w1T", [EDGE_DIM, H1], bf16, isOutput=False)
    w2_d = nc.declare_dram_parameter("w2T", [H1, H2], bf16, isOutput=False)
    w3_d = nc.declare_dram_parameter("w3T", [H2, F_OUT * F_IN], bf16, isOutput=False)
    sel_d = nc.declare_dram_parameter("sel", [128, 8 * F_OUT], bf16, isOutput=False)
    b3_d = nc.declare_dram_parameter("b3m", [F_IN, F_OUT], bf16, isOutput=False)
    b1_d = nc.declare_dram_parameter("b1v", [H1, 1], f32, isOutput=False)
    b2_d = nc.declare_dram_parameter("b2v", [H2, 1], f32, isOutput=False)
    out_d = nc.declare_dram_parameter("scan_out", [F_OUT, e_c], f32, isOutput=True)

    relu = mybir.ActivationFunctionType.Relu
    copy = mybir.ActivationFunctionType.Copy

    with tile.TileContext(nc) as tc:
        with (
            tc.tile_pool(name="const", bufs=1) as const,
            tc.tile_pool(name="io", bufs=3) as io,
            tc.tile_pool(name="mid", bufs=3) as mid,
            tc.tile_pool(name="scanb", bufs=3) as scanb,
            tc.tile_pool(name="psA", bufs=1, space="PSUM") as psA,
            tc.tile_pool(name="psB", bufs=2, space="PSUM") as psB,
            tc.tile_pool(name="psTH", bufs=2, space="PSUM") as psTH,
        ):
            s_w1 = const.tile([EDGE_DIM, H1], bf16)
            nc.sync.dma_start(out=s_w1, in_=w1_d[:])
            s_w2 = const.tile([H1, H2], bf16)
            nc.sync.dma_start(out=s_w2, in_=w2_d[:])
            s_w3 = const.tile([H2, F_OUT * F_IN], bf16)
            nc.sync.dma_start(out=s_w3, in_=w3_d[:])
            s_sel = const.tile([128, 8 * F_OUT], bf16)
            nc.sync.dma_start(out=s_sel, in_=sel_d[:])
            s_b3 = const.tile([F_IN, F_OUT], bf16)
            nc.sync.dma_start(out=s_b3, in_=b3_d[:])
            s_b1 = const.tile([H1, 1], f32)
            nc.sync.dma_start(out=s_b1, in_=b1_d[:])
            s_b2 = const.tile([H2, 1], f32)
            nc.sync.dma_start(out=s_b2, in_=b2_d[:])

            import contextlib

            loop_cm = (
                tc.For_i(
                    0,
                    bench_repeat,
                    1,
                    hint_engines=(
                        mybir.EngineType.PE,
                        mybir.EngineType.Activation,
                        mybir.EngineType.DVE,
                        mybir.EngineType.SP,
                        mybir.EngineType.Pool,
                    ),
                )
                if bench_repeat is not None
                else contextlib.nullcontext()
            )
            with loop_cm:
                prev_scan = None
                for t in range(n_tiles):
                        lo = t * E_TILE
                        hi = lo + E_TILE

                        ea_t = io.tile([EDGE_DIM, E_TILE], bf16, tag="ea")
                        nc.sync.dma_start(out=ea_t, in_=ea_t_d[:, lo:hi])
                        xs_t = io.tile([128, E_TILE], bf16, tag="xs")
                        nc.sync.dma_start(out=xs_t, in_=xs_d[:, lo:hi])
                        mk_t = io.tile([F_OUT, E_TILE], bf16, tag="mk")
                        nc.sync.dma_start(out=mk_t, in_=mk_d[:, lo:hi])

                        h1p = psA.tile([H1, E_TILE], f32, tag="h1")
                        nc.tensor.matmul(h1p, s_w1, ea_t, start=True, stop=True)
                        h1s = mid.tile([H1, E_TILE], bf16, tag="h1s")
                        nc.scalar.activation(h1s, h1p, relu, bias=s_b1)

                        h2p = psA.tile([H2, E_TILE], f32, tag="h2")
                        nc.tensor.matmul(h2p, s_w2, h1s, start=True, stop=True)
                        h2s = mid.tile([H2, E_TILE], bf16, tag="h2s")
                        nc.scalar.activation(h2s, h2p, relu, bias=s_b2)

                        msgp = psB.tile([F_OUT, E_TILE], f32, tag="msg")
                        if has_b3:
                            nc.tensor.matmul(
                                msgp, s_b3, xs_t[0:F_IN, :], start=True, stop=False
                            )
                        # theta blocks processed in PAIRS sharing a 2-bank PSUM
                        # tile.  Pairs 0-2: one ACT evac [128,2*512] f32->bf16,
                        # then one DVE multiply per pair (xs broadcast via a
                        # stride-0 middle dim).  Pair 3: DVE multiplies straight
                        # from PSUM.  Balances ACT vs DVE occupancy.
                        xs2 = bass.AP(
                            tensor=xs_t.tensor,
                            offset=xs_t.offset,
                            ap=[list(xs_t.ap[0]), [0, 2], list(xs_t.ap[1])],
                        )
                        for p in range(4):
                            thp2 = psTH.tile([128, 2, E_TILE], f32, tag="th")
                            for h in range(2):
                                b = 2 * p + h
                                nc.tensor.matmul(
                                    thp2[:, h, :],
                                    s_w3[:, b * 128 : (b + 1) * 128],
                                    h2s,
                                    start=True,
                                    stop=True,
                                )
                            prod2 = mid.tile([128, 2, E_TILE], bf16, tag="prod")
                            if p == 3:
                                nc.vector.tensor_mul(prod2, thp2, xs2)
                            else:
                                ths2 = mid.tile([128, 2, E_TILE], bf16, tag="ths")
                                nc.scalar.activation(ths2, thp2, copy)
                                nc.vector.tensor_mul(prod2, ths2, xs2)
                            for h in range(2):
                                b = 2 * p + h
                                nc.tensor.matmul(
                                    msgp,
                                    s_sel[:, b * F_OUT : (b + 1) * F_OUT],
                                    prod2[:, h, :],
                                    start=(b == 0 and not has_b3),
                                    stop=(b == 7),
                                )

                        sc = scanb.tile([F_OUT, E_TILE], f32, tag="scan")
                        initial = 0.0 if prev_scan is None else prev_scan[:, E_TILE - 1 : E_TILE]
                        nc.vector.tensor_tensor_scan(
                            sc,
                            mk_t,
                            msgp,
                            initial=initial,
                            op0=mybir.AluOpType.mult,
                            op1=mybir.AluOpType.add,
                        )
                        prev_scan = sc
                        nc.sync.dma_start(out=out_d[:, lo:hi], in_=sc)

    nc.finalize()
    return nc


def kernel(x, edge_attr, W1, b1, W2, b2, W3, b3, edge_src, edge_dst):
    x = np.asarray(x, dtype=np.float32)
    edge_attr = np.asarray(edge_attr, dtype=np.float32)
    W1 = np.asarray(W1, dtype=np.float32)
    b1 = np.asarray(b1, dtype=np.float32)
    W2 = np.asarray(W2, dtype=np.float32)
    b2 = np.asarray(b2, dtype=np.float32)
    W3 = np.asarray(W3, dtype=np.float32)
    b3 = np.asarray(b3, dtype=np.float32)
    edge_src = np.asarray(edge_src).astype(np.int64)
    edge_dst = np.asarray(edge_dst).astype(np.int64)

    n_nodes = x.shape[0]
    n_edges = edge_dst.shape[0]

    # ---- host preprocessing: sort by destination, shard on segment bounds
    order = np.argsort(edge_dst, kind="stable")
    dst_s = edge_dst[order]
    src_s = edge_src[order]
    ea_s = edge_attr[order]

    cuts = [0]
    for c in range(1, N_CORES):
        t = c * n_edges // N_CORES
        while t < n_edges and dst_s[t] == dst_s[t - 1]:
            t += 1
        cuts.append(min(t, n_edges))
    cuts.append(n_edges)
    counts = [cuts[i + 1] - cuts[i] for i in range(N_CORES)]
    e_c = max(E_TILE, int(math.ceil(max(counts) / E_TILE)) * E_TILE)

    deg = np.bincount(edge_dst, minlength=n_nodes).astype(np.float32)
    inv_deg = 1.0 / np.maximum(deg, 1.0)

    # ---- shared weight payloads
    w1T = np.ascontiguousarray(W1.T).astype(BF16)                  # [6, 64]
    w2T = np.ascontiguousarray(W2.T).astype(BF16)                  # [64, 128]
    w3T = np.ascontiguousarray(W3.T).astype(BF16)                  # [128, 1024]
    b3m = np.ascontiguousarray(b3.reshape(F_OUT, F_IN).T).astype(BF16)
    b1v = b1.reshape(H1, 1).astype(np.float32)
    b2v = b2.reshape(H2, 1).astype(np.float32)
    sel = np.zeros((128, 8 * F_OUT), dtype=np.float32)
    rows = np.arange(128)
    for b in range(8):
        sel[rows, b * F_OUT + (4 * b + rows // 32)] = 1.0
    sel = sel.astype(BF16)

    in_maps = []
    core_meta = []
    for c in range(N_CORES):
        lo, hi = cuts[c], cuts[c + 1]
        cnt = hi - lo
        dst_c = dst_s[lo:hi]
        xs_c = x[src_s[lo:hi]]                                     # [cnt, 32]

        ea_pad = np.zeros((e_c, EDGE_DIM), dtype=np.float32)
        ea_pad[:cnt] = ea_s[lo:hi]
        xs_pad = np.zeros((e_c, F_IN), dtype=np.float32)
        xs_pad[:cnt] = xs_c
        keep = np.zeros(e_c, dtype=np.float32)
        if cnt > 1:
            keep[1:cnt] = (dst_c[1:] == dst_c[:-1]).astype(np.float32)

        eaT = np.ascontiguousarray(ea_pad.T).astype(BF16)          # [6, e_c]
        xsT = np.ascontiguousarray(xs_pad.T)                       # [32, e_c]
        xsrep = np.tile(xsT, (4, 1)).astype(BF16)                  # [128, e_c]
        mask = np.broadcast_to(keep, (F_OUT, e_c)).astype(BF16)

        # last index of each segment in this shard
        if cnt > 0:
            is_end = np.empty(cnt, dtype=bool)
            is_end[-1] = True
            is_end[:-1] = dst_c[1:] != dst_c[:-1]
            ends = np.flatnonzero(is_end)
            nodes = dst_c[ends]
        else:
            ends = np.zeros(0, dtype=np.int64)
            nodes = np.zeros(0, dtype=np.int64)
        core_meta.append((ends, nodes))

        in_maps.append(
            {
                "eaT": eaT,
                "xsrep": xsrep,
                "mask": np.ascontiguousarray(mask),
                "w1T": w1T,
                "w2T": w2T,
                "w3T": w3T,
                "sel": sel,
                "b3m": b3m,
                "b1v": b1v,
                "b2v": b2v,
            }
        )

    has_b3 = bool(np.any(b3))
    key = (e_c, has_b3)
    if key not in _program_cache:
        _program_cache[key] = _build_program(e_c, has_b3=has_b3)
    nc = _program_cache[key]

    res = run_bass_kernel_spmd(nc, in_maps, list(range(N_CORES)))

    out = np.zeros((n_nodes, F_OUT), dtype=np.float32)
    for c in range(N_CORES):
        scan = np.asarray(res.results[c]["scan_out"], dtype=np.float32)
        ends, nodes = core_meta[c]
        if len(nodes):
            out[nodes] = scan[:, ends].T * inv_deg[nodes, None]
    np.maximum(out, 0.0, out=out)
    return out



# revision 3
# speedup vs baseline: 1.4560x; 1.4560x over previous
"""ECC (edge-conditioned convolution) GNN message passing on 8 NeuronCores.

Strategy
--------
Edges are sorted by destination node (host side) and split into 8
contiguous, segment-aligned shards -- one per core.  Each core runs an
identical Bass program over its shard:

  PE    : h1 = relu(W1 @ eaT); h2 = relu(W2 @ h1);
          theta blocks (8x [128,512] per 512-edge tile) = W3T_b.T @ h2;
          b3 term + selector matmuls reduce over i:
             msg[o,e] = sum_i theta[(o,i),e] * xs[i,e]
  ACT   : PSUM->SBUF evacuation (relu / f32->bf16 casts)
  DVE   : theta * xs_rep elementwise (bf16 2x mode) and a masked prefix
          scan (tensor_tensor_scan) that produces running segment sums.

The scan output [32, E_c] goes back to HBM; the host reads each segment's
last column (positions are known statically from the sort), divides by
degree and applies the final relu.  Because shards are segment aligned no
cross-core reduction is needed.
"""

import math
import sys

import numpy as np

for _p in ("/opt/trn_rl_repo", "/root/.axon_site/_ro/trn_rl_repo"):
    if _p not in sys.path:
        sys.path.insert(0, _p)

import ml_dtypes

import concourse.bass as bass
import concourse.mybir as mybir
import concourse.tile as tile
from concourse import bacc
from concourse.bass_utils import run_bass_kernel_spmd

N_NODES = 25000
N_EDGES = 250000
F_IN = 32
F_OUT = 32
EDGE_DIM = 6
H1, H2 = 64, 128
N_CORES = 8
E_TILE = 512

BF16 = ml_dtypes.bfloat16

_program_cache: dict = {}


def _build_program(
    e_c: int, bench_repeat: int | None = None, has_b3: bool = True
) -> "bass.Bass":
    f32 = mybir.dt.float32
    bf16 = mybir.dt.bfloat16
    n_tiles = e_c // E_TILE

    nc = bacc.Bacc(None, target_bir_lowering=False)

    ea_t_d = nc.declare_dram_parameter("eaT", [EDGE_DIM, e_c], bf16, isOutput=False)
    xs_d = nc.declare_dram_parameter("xsrep", [128, e_c], bf16, isOutput=False)
    mk_d = nc.declare_dram_parameter("mask", [F_OUT, e_c], bf16, isOutput=False)
    w1_d = nc.declare_dram_parameter("w1T", [EDGE_DIM, H1], bf16, isOutput=False)
    w2_d = nc.declare_dram_parameter("w2T", [H1, H2], bf16, isOutput=False)
    w3_d = nc.declare_dram_parameter("w3T", [H2, F_OUT * F_IN], bf16, isOutput=False)
    sel_d = nc.declare_dram_parameter("sel", [128, 8 * F_OUT], bf16, isOutput=False)
    b3_d = nc.declare_dram_parameter("b3m", [F_IN, F_OUT], bf16, isOutput=False)
    b1_d = nc.declare_dram_parameter("b1v", [H1, 1], f32, isOutput=False)
    b2_d = nc.declare_dram_parameter("b2v", [H2, 1], f32, isOutput=False)
    out_d = nc.declare_dram_parameter("scan_out", [F_OUT, e_c], f32, isOutput=True)

    relu = mybir.ActivationFunctionType.Relu
    copy = mybir.ActivationFunctionType.Copy

    with tile.TileContext(nc) as tc:
        with (
            tc.tile_pool(name="const", bufs=1) as const,
            tc.tile_pool(name="io", bufs=3) as io,
            tc.tile_pool(name="mid", bufs=3) as mid,
            tc.tile_pool(name="scanb", bufs=3) as scanb,
            tc.tile_pool(name="psA", bufs=1, space="PSUM") as psA,
            tc.tile_pool(name="psB", bufs=2, space="PSUM") as psB,
            tc.tile_pool(name="psTH", bufs=2, space="PSUM") as psTH,
        ):
            s_w1 = const.tile([EDGE_DIM, H1], bf16)
            nc.sync.dma_start(out=s_w1, in_=w1_d[:])
            s_w2 = const.tile([H1, H2], bf16)
            nc.sync.dma_start(out=s_w2, in_=w2_d[:])
            s_w3 = const.tile([H2, F_OUT * F_IN], bf16)
            nc.sync.dma_start(out=s_w3, in_=w3_d[:])
            s_sel = const.tile([128, 8 * F_OUT], bf16)
            nc.sync.dma_start(out=s_sel, in_=sel_d[:])
            s_b3 = const.tile([F_IN, F_OUT], bf16)
            nc.sync.dma_start(out=s_b3, in_=b3_d[:])
            s_b1 = const.tile([H1, 1], f32)
            nc.sync.dma_start(out=s_b1, in_=b1_d[:])
            s_b2 = const.tile([H2, 1], f32)
            nc.sync.dma_start(out=s_b2, in_=b2_d[:])

            import contextlib

            loop_cm = (
                tc.For_i(
                    0,
                    bench_repeat,
                    1,
                    hint_engines=(
                        mybir.EngineType.PE,
                        mybir.EngineType.Activation,
                        mybir.EngineType.DVE,
                        mybir.EngineType.SP,
                        mybir.EngineType.Pool,
                    ),
                )
                if bench_repeat is not None
                else contextlib.nullcontext()
            )
            with loop_cm:
                prev_scan = None
                for t in range(n_tiles):
                        lo = t * E_TILE
                        hi = lo + E_TILE

                        ea_t = io.tile([EDGE_DIM, E_TILE], bf16, tag="ea")
                        nc.sync.dma_start(out=ea_t, in_=ea_t_d[:, lo:hi])
                        xs_t = io.tile([128, E_TILE], bf16, tag="xs")
                        nc.sync.dma_start(out=xs_t, in_=xs_d[:, lo:hi])
                        mk_t = io.tile([F_OUT, E_TILE], bf16, tag="mk")
                        nc.sync.dma_start(out=mk_t, in_=mk_d[:, lo:hi])

                        h1p = psA.tile([H1, E_TILE], f32, tag="h1")
                        nc.tensor.matmul(h1p, s_w1, ea_t, start=True, stop=True)
                        h1s = mid.tile([H1, E_TILE], bf16, tag="h1s")
                        nc.scalar.activation(h1s, h1p, relu, bias=s_b1)

                        h2p = psA.tile([H2, E_TILE], f32, tag="h2")
                        nc.tensor.matmul(h2p, s_w2, h1s, start=True, stop=True)
                        h2s = mid.tile([H2, E_TILE], bf16, tag="h2s")
                        nc.scalar.activation(h2s, h2p, relu, bias=s_b2)

                        msgp = psB.tile([F_OUT, E_TILE], f32, tag="msg")
                        if has_b3:
                            nc.tensor.matmul(
                                msgp, s_b3, xs_t[0:F_IN, :], start=True, stop=False
                            )
                        # theta blocks processed in PAIRS sharing a 2-bank PSUM
                        # tile.  Pairs 0-2: one ACT evac [128,2*512] f32->bf16,
                        # then one DVE multiply per pair (xs broadcast via a
                        # stride-0 middle dim).  Pair 3: DVE multiplies straight
                        # from PSUM.  Balances ACT vs DVE occupancy.
                        xs2 = bass.AP(
                            tensor=xs_t.tensor,
                            offset=xs_t.offset,
                            ap=[list(xs_t.ap[0]), [0, 2], list(xs_t.ap[1])],
                        )
                        for p in range(4):
                            thp2 = psTH.tile([128, 2, E_TILE], f32, tag="th")
                            for h in range(2):
                                b = 2 * p + h
                                nc.tensor.matmul(
                                    thp2[:, h, :],
                                    s_w3[:, b * 128 : (b + 1) * 128],
                                    h2s,
                                    start=True,
                                    stop=True,
                                )
                            prod2 = mid.tile([128, 2, E_TILE], bf16, tag="prod")
                            if p == 3:
                                nc.vector.tensor_mul(prod2, thp2, xs2)
                            else:
                                ths2 = mid.tile([128, 2, E_TILE], bf16, tag="ths")
                                nc.scalar.activation(ths2, thp2, copy)
                                nc.vector.tensor_mul(prod2, ths2, xs2)
                            for h in range(2):
                                b = 2 * p + h
                                nc.tensor.matmul(
                                    msgp,
                                    s_sel[:, b * F_OUT : (b + 1) * F_OUT],
                                    prod2[:, h, :],
                                    start=(b == 0 and not has_b3),
                                    stop=(b == 7),
                                )

                        sc = scanb.tile([F_OUT, E_TILE], f32, tag="scan")
                        initial = 0.0 if prev_scan is None else prev_scan[:, E_TILE - 1 : E_TILE]
                        nc.vector.tensor_tensor_scan(
                            sc,
                            mk_t,
                            msgp,
                            initial=initial,
                            op0=mybir.AluOpType.mult,
                            op1=mybir.AluOpType.add,
                        )
                        prev_scan = sc
                        nc.sync.dma_start(out=out_d[:, lo:hi], in_=sc)

    nc.finalize()
    return nc


def _prepare(x, edge_attr, W1, b1, W2, b2, W3, b3, edge_src, edge_dst):
    x = np.asarray(x, dtype=np.float32)
    edge_attr = np.asarray(edge_attr, dtype=np.float32)
    W1 = np.asarray(W1, dtype=np.float32)
    b1 = np.asarray(b1, dtype=np.float32)
    W2 = np.asarray(W2, dtype=np.float32)
    b2 = np.asarray(b2, dtype=np.float32)
    W3 = np.asarray(W3, dtype=np.float32)
    b3 = np.asarray(b3, dtype=np.float32)
    edge_src = np.asarray(edge_src).astype(np.int64)
    edge_dst = np.asarray(edge_dst).astype(np.int64)

    n_nodes = x.shape[0]
    n_edges = edge_dst.shape[0]

    # ---- host preprocessing: sort by destination, shard on segment bounds
    order = np.argsort(edge_dst, kind="stable")
    dst_s = edge_dst[order]
    src_s = edge_src[order]
    ea_s = edge_attr[order]

    cuts = [0]
    for c in range(1, N_CORES):
        t = c * n_edges // N_CORES
        while t < n_edges and dst_s[t] == dst_s[t - 1]:
            t += 1
        cuts.append(min(t, n_edges))
    cuts.append(n_edges)
    counts = [cuts[i + 1] - cuts[i] for i in range(N_CORES)]
    e_c = max(E_TILE, int(math.ceil(max(counts) / E_TILE)) * E_TILE)

    deg = np.bincount(edge_dst, minlength=n_nodes).astype(np.float32)
    inv_deg = 1.0 / np.maximum(deg, 1.0)

    # ---- shared weight payloads
    w1T = np.ascontiguousarray(W1.T).astype(BF16)                  # [6, 64]
    w2T = np.ascontiguousarray(W2.T).astype(BF16)                  # [64, 128]
    w3T = np.ascontiguousarray(W3.T).astype(BF16)                  # [128, 1024]
    b3m = np.ascontiguousarray(b3.reshape(F_OUT, F_IN).T).astype(BF16)
    b1v = b1.reshape(H1, 1).astype(np.float32)
    b2v = b2.reshape(H2, 1).astype(np.float32)
    sel = np.zeros((128, 8 * F_OUT), dtype=np.float32)
    rows = np.arange(128)
    for b in range(8):
        sel[rows, b * F_OUT + (4 * b + rows // 32)] = 1.0
    sel = sel.astype(BF16)

    in_maps = []
    core_meta = []
    for c in range(N_CORES):
        lo, hi = cuts[c], cuts[c + 1]
        cnt = hi - lo
        dst_c = dst_s[lo:hi]
        xs_c = x[src_s[lo:hi]]                                     # [cnt, 32]

        ea_pad = np.zeros((e_c, EDGE_DIM), dtype=np.float32)
        ea_pad[:cnt] = ea_s[lo:hi]
        xs_pad = np.zeros((e_c, F_IN), dtype=np.float32)
        xs_pad[:cnt] = xs_c
        keep = np.zeros(e_c, dtype=np.float32)
        if cnt > 1:
            keep[1:cnt] = (dst_c[1:] == dst_c[:-1]).astype(np.float32)

        eaT = np.ascontiguousarray(ea_pad.T).astype(BF16)          # [6, e_c]
        xsT = np.ascontiguousarray(xs_pad.T)                       # [32, e_c]
        xsrep = np.tile(xsT, (4, 1)).astype(BF16)                  # [128, e_c]
        mask = np.broadcast_to(keep, (F_OUT, e_c)).astype(BF16)

        # last index of each segment in this shard
        if cnt > 0:
            is_end = np.empty(cnt, dtype=bool)
            is_end[-1] = True
            is_end[:-1] = dst_c[1:] != dst_c[:-1]
            ends = np.flatnonzero(is_end)
            nodes = dst_c[ends]
        else:
            ends = np.zeros(0, dtype=np.int64)
            nodes = np.zeros(0, dtype=np.int64)
        core_meta.append((ends, nodes))

        in_maps.append(
            {
                "eaT": eaT,
                "xsrep": xsrep,
                "mask": np.ascontiguousarray(mask),
                "w1T": w1T,
                "w2T": w2T,
                "w3T": w3T,
                "sel": sel,
                "b3m": b3m,
                "b1v": b1v,
                "b2v": b2v,
            }
        )

    has_b3 = bool(np.any(b3))
    return {
        "in_maps": in_maps,
        "core_meta": core_meta,
        "inv_deg": inv_deg,
        "e_c": e_c,
        "has_b3": has_b3,
        "n_nodes": n_nodes,
    }


def _postprocess(res, meta):
    out = np.zeros((meta["n_nodes"], F_OUT), dtype=np.float32)
    inv_deg = meta["inv_deg"]
    for c in range(N_CORES):
        scan = np.asarray(res.results[c]["scan_out"], dtype=np.float32)
        ends, nodes = meta["core_meta"][c]
        if len(nodes):
            out[nodes] = scan[:, ends].T * inv_deg[nodes, None]
    np.maximum(out, 0.0, out=out)
    return out


def kernel(x, edge_attr, W1, b1, W2, b2, W3, b3, edge_src, edge_dst):
    meta = _prepare(x, edge_attr, W1, b1, W2, b2, W3, b3, edge_src, edge_dst)
    key = (meta["e_c"], meta["has_b3"])
    if key not in _program_cache:
        _program_cache[key] = _build_program(meta["e_c"], has_b3=meta["has_b3"])
    nc = _program_cache[key]

    res = run_bass_kernel_spmd(nc, meta["in_maps"], list(range(N_CORES)))
    return _postprocess(res, meta)

